# revision 7
# baseline (speedup 1.0000x reference)
"""Trainium2 Bass kernel: y = x @ W.T + b  (fp16 Linear, B=4 S=2048 D=4096).

Sharding: pure data-parallel over the 8192 token rows -> 8 NeuronCores,
1024 tokens each. No collectives needed; each core runs an independent
(1024x4096) @ (4096x4096) GEMM + bias.

Self-contained: hardcodes shapes; builds/compiles the Bass graph once per
process and runs it via run_bass_kernel_spmd on cores 0-7.
"""

import os
from contextlib import ExitStack

import numpy as np

import concourse.bass as bass
import concourse.tile as tile
from concourse import bacc, mybir
from concourse.bass_utils import run_bass_kernel_spmd
from concourse.kernels.tile_matmul import matmul_tile_kernel

B, S, D_IN, D_OUT = 4, 2048, 4096, 4096
N_CORES = 8
S_TOTAL = B * S  # 8192 token rows
S_SHARD = S_TOTAL // N_CORES  # 1024 rows per core

# Set by test harness: when truthy, run with NTFF tracing and stash the
# BassKernelResults (incl. exec_time_ns) in LAST_RESULTS.
TRACE = False
TRACE_DIR = None
LAST_RESULTS = None

_CACHE = {}


def _build_v2(chain=1):
    """Custom loop: X.T resident in SBUF, W.T streamed once (n-outer),
    8 PSUM banks = 8 token-tiles, bias fused into PSUM eviction."""
    nc = bacc.Bacc(
        "TRN2",
        target_bir_lowering=False,
        debug=False,
        num_devices=N_CORES,
    )
    x = nc.dram_tensor("input", [S_SHARD, D_IN], mybir.dt.float16, kind="ExternalInput")
    w = nc.dram_tensor("weight", [D_OUT, D_IN], mybir.dt.float16, kind="ExternalInput")
    b = nc.dram_tensor("bias", [1, D_OUT], mybir.dt.float16, kind="ExternalInput")
    y = nc.dram_tensor("out", [S_SHARD, D_OUT], mybir.dt.float16, kind="ExternalOutput")

    KT = D_IN // 128  # 32 k-tiles
    MT = S_SHARD // 128  # 8 token-tiles
    NT = D_OUT // 512  # 8 out-feature tiles

    w_re = w.ap().rearrange("f (po pi) -> f po pi", pi=128)  # [4096 o, 32, 128]
    y_re = y.ap().rearrange("(po pi) f -> pi po f", pi=128)  # [128, 8, 4096]

    with tile.TileContext(nc) as tc, ExitStack() as ctx:
        const_pool = ctx.enter_context(tc.tile_pool(name="const", bufs=1))
        xT_pool = ctx.enter_context(tc.tile_pool(name="xT", bufs=1))
        wT_pool = ctx.enter_context(tc.tile_pool(name="wT", bufs=3))
        out_pool = ctx.enter_context(tc.tile_pool(name="outp", bufs=4))
        psum_pool = ctx.enter_context(tc.tile_pool(name="ps", bufs=1, space="PSUM"))

        bias_rep = const_pool.tile([128, D_OUT], mybir.dt.float16)
        nc.sync.dma_start(bias_rep[:], b.ap().to_broadcast((128, D_OUT)))

        for it in range(chain):
            if it > 0:
                tc.strict_bb_all_engine_barrier()
            # X.T resident: [128 k-inner, 32 k-outer, 1024 tokens]
            xT = xT_pool.tile([128, KT, S_SHARD], mybir.dt.float16, name=f"xT{it}", tag="xT")
            for k in range(KT):
                nc.sync.dma_start(
                    xT[:, k, :], x.ap()[:, bass.ds(k * 128, 128)], transpose=True
                )
            for n in range(NT):
                wT = wT_pool.tile([128, KT, 512], mybir.dt.float16, name=f"wT{it}_{n}", tag="wT")
                nc.sync.dma_start_transpose(
                    wT[:], w_re[bass.ds(n * 512, 512), :].opt(keep_dims=frozenset({0}))
                )
                pss = [
                    psum_pool.tile(
                        [128, 512], mybir.dt.float32, name=f"ps{it}_{n}_{m}", tag=f"ps{m}"
                    )
                    for m in range(MT)
                ]
                for k in range(KT):
                    for m in range(MT):
                        nc.tensor.matmul(
                            pss[m][:],
                            xT[:, k, bass.ds(m * 128, 128)],
                            wT[:, k, :],
                            start=(k == 0),
                            stop=(k == KT - 1),
                        )
                for m in range(MT):
                    ot = out_pool.tile([128, 512], mybir.dt.float16, name=f"ot{it}_{n}_{m}", tag="ot")
                    nc.vector.tensor_tensor(
                        ot[:],
                        pss[m][:],
                        bias_rep[:, bass.ds(n * 512, 512)],
                        mybir.AluOpType.add,
                    )
                    nc.sync.dma_start(y_re[:, m, bass.ds(n * 512, 512)], ot[:])

    nc.compile()
    return nc


def _build(chain=1):
    nc = bacc.Bacc(
        "TRN2",
        target_bir_lowering=False,
        debug=False,
        num_devices=N_CORES,
    )
    x = nc.dram_tensor("input", [S_SHARD, D_IN], mybir.dt.float16, kind="ExternalInput")
    w = nc.dram_tensor("weight", [D_OUT, D_IN], mybir.dt.float16, kind="ExternalInput")
    b = nc.dram_tensor("bias", [1, D_OUT], mybir.dt.float16, kind="ExternalInput")
    y = nc.dram_tensor("out", [S_SHARD, D_OUT], mybir.dt.float16, kind="ExternalOutput")

    with tile.TileContext(nc) as tc:
        with tc.tile_pool(name="bias_pool", bufs=1) as bias_pool:
            bias_rep = bias_pool.tile([128, D_OUT], mybir.dt.float16)
            # Replicate the bias row into all 128 partitions once.
            nc.sync.dma_start(bias_rep[:], b.ap().to_broadcast((128, D_OUT)))

            def add_bias(nc_, sbuf, md, _data):
                start = md.n_tile_idx * md.n_tile
                nsz = sbuf.shape[-1]
                nc_.vector.tensor_tensor(
                    sbuf,
                    sbuf,
                    bias_rep[:, None, start : start + nsz].to_broadcast(sbuf.shape),
                    mybir.AluOpType.add,
                )

            for it in range(chain):
                if it > 0:
                    # Serialize benchmark iterations so T(chain)-T(1) is an
                    # honest per-execution time.
                    tc.strict_bb_all_engine_barrier()
                matmul_tile_kernel(
                    tc,
                    x.ap(),  # kxm: X.T via DMA transpose -> psum partitions = tokens
                    w.ap(),  # kxn: W.T via DMA transpose -> free dim = out_features
                    y.ap(),
                    transpose_kxm=True,
                    transpose_kxn=True,
                    post_mxn_tile_fn=add_bias,
                    MAX_K_TILE_SIZE=4096,
                )

    nc.compile()
    return nc


def _make_sharded(nc):
    """Build a jitted shard_map fn executing nc's NEFF once on 8 cores.
    Returns (fn, in_names, out_names, out_avals, n_params)."""
    import jax
    from jax.sharding import Mesh, PartitionSpec
    from jax.experimental.shard_map import shard_map

    from concourse import bass2jax, mybir as _mybir

    bass2jax.install_neuronx_cc_hook()

    partition_name = nc.partition_id_tensor.name if nc.partition_id_tensor else None
    in_names, out_names, out_avals = [], [], []
    for alloc in nc.m.functions[0].allocations:
        if not isinstance(alloc, _mybir.MemoryLocationSet):
            continue
        name = alloc.memorylocations[0].name
        if alloc.kind == "ExternalInput":
            if name != partition_name:
                in_names.append(name)
        elif alloc.kind == "ExternalOutput":
            out_names.append(name)
            shape = tuple(alloc.tensor_shape)
            dtype = _mybir.dt.np(alloc.dtype)
            out_avals.append(jax.core.ShapedArray(shape, dtype))
    n_params = len(in_names)
    all_in_names = in_names + out_names
    if partition_name is not None:
        all_in_names.append(partition_name)

    def _body(*args):
        operands = list(args)
        if partition_name is not None:
            operands.append(bass2jax.partition_id_tensor())
        return tuple(
            bass2jax._bass_exec_p.bind(
                *operands,
                out_avals=tuple(out_avals),
                in_names=tuple(all_in_names),
                out_names=tuple(out_names),
                lowering_input_output_aliases=(),
                sim_require_finite=True,
                sim_require_nnan=True,
                nc=nc,
            )
        )

    devices = jax.devices()[:N_CORES]
    mesh = Mesh(np.asarray(devices), ("core",))
    n_outs = len(out_names)
    in_specs = (PartitionSpec("core"),) * (n_params + n_outs)
    out_specs = (PartitionSpec("core"),) * n_outs
    fn = jax.jit(
        shard_map(
            _body, mesh=mesh, in_specs=in_specs, out_specs=out_specs, check_rep=False
        ),
        keep_unused=True,
    )
    return fn, in_names, out_names, out_avals, n_params


BUILDER = None  # set after definitions; defaults to _build_v2


def benchmark(input, weight, bias, iters=12, reps=6):
    """Marginal per-GEMM time: compares a NEFF containing `iters` chained
    (barrier-separated) copies of the kernel against the 1-copy NEFF.
    per_exec = (T(iters) - T(1)) / (iters - 1), min over reps.
    Returns (per_exec_seconds, outputs_list from the 1-copy NEFF).
    """
    import time

    import jax
    from jax.sharding import Mesh, NamedSharding, PartitionSpec

    bld = BUILDER or _build_v2
    if "nc" not in _CACHE:
        _CACHE["nc"] = bld()
    nc1 = _CACHE["nc"]
    key = f"nc_chain{iters}"
    if key not in _CACHE:
        _CACHE[key] = bld(chain=iters)
    ncK = _CACHE[key]

    X = np.ascontiguousarray(np.asarray(input, dtype=np.float16).reshape(S_TOTAL, D_IN))
    Wm = np.ascontiguousarray(np.asarray(weight, dtype=np.float16))
    bm = np.ascontiguousarray(np.asarray(bias, dtype=np.float16).reshape(1, D_OUT))
    in_maps = [
        {
            "input": np.ascontiguousarray(X[i * S_SHARD : (i + 1) * S_SHARD]),
            "weight": Wm,
            "bias": bm,
        }
        for i in range(N_CORES)
    ]

    fn1, in_names, out_names, out_avals, n_params = _make_sharded(nc1)
    fnK = _make_sharded(ncK)[0]

    concat_in = [
        np.concatenate([np.asarray(in_maps[c][nm]) for c in range(N_CORES)], axis=0)
        for nm in in_names
    ]
    concat_zeros = [
        np.zeros((N_CORES * a.shape[0], *a.shape[1:]), a.dtype) for a in out_avals
    ]
    mesh = Mesh(np.asarray(jax.devices()[:N_CORES]), ("core",))
    sh = NamedSharding(mesh, PartitionSpec("core"))
    dev_in = [jax.device_put(a, sh) for a in concat_in]
    dev_zero = [jax.device_put(a, sh) for a in concat_zeros]

    # Warmup both (compiles wrapper + NEFF)
    outs = fn1(*dev_in, *dev_zero)
    jax.block_until_ready(outs)
    outsK = fnK(*dev_in, *dev_zero)
    jax.block_until_ready(outsK)

    best = float("inf")
    t1s, tKs = [], []
    for _ in range(reps):
        t0 = time.perf_counter()
        o1 = fn1(*dev_in, *dev_zero)
        jax.block_until_ready(o1)
        t1 = time.perf_counter()
        oK = fnK(*dev_in, *dev_zero)
        jax.block_until_ready(oK)
        t2 = time.perf_counter()
        t1s.append(t1 - t0)
        tKs.append(t2 - t1)
    per_exec = (min(tKs) - min(t1s)) / (iters - 1)
    print(f"[bench] T1 min={min(t1s)*1e3:.3f} ms  TK min={min(tKs)*1e3:.3f} ms  "
          f"(iters={iters})")

    out_np = [np.asarray(o) for o in outs]
    results = [
        {
            nm: out_np[i].reshape(N_CORES, *out_avals[i].shape)[c]
            for i, nm in enumerate(out_names)
        }
        for c in range(N_CORES)
    ]
    return per_exec, results


def kernel(input, weight, bias):
    global LAST_RESULTS
    bld = BUILDER or _build_v2
    if "nc" not in _CACHE:
        _CACHE["nc"] = bld()
    nc = _CACHE["nc"]

    X = np.ascontiguousarray(np.asarray(input, dtype=np.float16).reshape(S_TOTAL, D_IN))
    Wm = np.ascontiguousarray(np.asarray(weight, dtype=np.float16))
    bm = np.ascontiguousarray(np.asarray(bias, dtype=np.float16).reshape(1, D_OUT))

    in_maps = [
        {
            "input": np.ascontiguousarray(X[i * S_SHARD : (i + 1) * S_SHARD]),
            "weight": Wm,
            "bias": bm,
        }
        for i in range(N_CORES)
    ]

    kwargs = {}
    if TRACE:
        kwargs = dict(trace=True, tmpdir=TRACE_DIR)
    res = run_bass_kernel_spmd(nc, in_maps, list(range(N_CORES)), **kwargs)
    LAST_RESULTS = res

    Y = np.concatenate([res.results[i]["out"] for i in range(N_CORES)], axis=0)
    return Y.reshape(B, S, D_OUT)


# revision 8
# speedup vs baseline: 2.0615x; 2.0615x over previous
"""Trainium2 Bass kernel: y = x @ W.T + b  (fp16 Linear, B=4 S=2048 D=4096).

Sharding: pure data-parallel over the 8192 token rows -> 8 NeuronCores,
1024 tokens each. No collectives needed; each core runs an independent
(1024x4096) @ (4096x4096) GEMM + bias.

Self-contained: hardcodes shapes; builds/compiles the Bass graph once per
process and runs it via run_bass_kernel_spmd on cores 0-7.
"""

import os
from contextlib import ExitStack

import numpy as np

import concourse.bass as bass
import concourse.tile as tile
from concourse import bacc, mybir
from concourse.bass_utils import run_bass_kernel_spmd
from concourse.kernels.tile_matmul import matmul_tile_kernel

B, S, D_IN, D_OUT = 4, 2048, 4096, 4096
N_CORES = 8
S_TOTAL = B * S  # 8192 token rows
S_SHARD = S_TOTAL // N_CORES  # 1024 rows per core

# Set by test harness: when truthy, run with NTFF tracing and stash the
# BassKernelResults (incl. exec_time_ns) in LAST_RESULTS.
TRACE = False
TRACE_DIR = None
LAST_RESULTS = None

_CACHE = {}


def _build_v2(chain=1):
    """Custom loop: X.T resident in SBUF, W.T streamed once (n-outer),
    8 PSUM banks = 8 token-tiles, bias fused into PSUM eviction."""
    nc = bacc.Bacc(
        "TRN2",
        target_bir_lowering=False,
        debug=False,
        num_devices=N_CORES,
    )
    x = nc.dram_tensor("input", [S_SHARD, D_IN], mybir.dt.float16, kind="ExternalInput")
    w = nc.dram_tensor("weight", [D_OUT, D_IN], mybir.dt.float16, kind="ExternalInput")
    b = nc.dram_tensor("bias", [1, D_OUT], mybir.dt.float16, kind="ExternalInput")
    y = nc.dram_tensor("out", [S_SHARD, D_OUT], mybir.dt.float16, kind="ExternalOutput")

    KT = D_IN // 128  # 32 k-tiles
    MT = S_SHARD // 128  # 8 token-tiles
    NT = D_OUT // 512  # 8 out-feature tiles

    w_re = w.ap().rearrange("f (po pi) -> f po pi", pi=128)  # [4096 o, 32, 128]
    y_re = y.ap().rearrange("(po pi) f -> pi po f", pi=128)  # [128, 8, 4096]

    with tile.TileContext(nc) as tc, ExitStack() as ctx:
        const_pool = ctx.enter_context(tc.tile_pool(name="const", bufs=1))
        xT_pool = ctx.enter_context(tc.tile_pool(name="xT", bufs=1))
        wT_pool = ctx.enter_context(tc.tile_pool(name="wT", bufs=3))
        out_pool = ctx.enter_context(tc.tile_pool(name="outp", bufs=4))
        psum_pool = ctx.enter_context(tc.tile_pool(name="ps", bufs=1, space="PSUM"))

        bias_rep = const_pool.tile([128, D_OUT], mybir.dt.float16)
        nc.sync.dma_start(bias_rep[:], b.ap().to_broadcast((128, D_OUT)))

        for it in range(chain):
            if it > 0:
                tc.strict_bb_all_engine_barrier()
            # X.T resident: [128 k-inner, 32 k-outer, 1024 tokens], one big
            # contiguous-source XBAR transpose of the whole shard.
            xT = xT_pool.tile([128, KT, S_SHARD], mybir.dt.float16, name=f"xT{it}", tag="xT")
            nc.sync.dma_start_transpose(
                xT[:], x.ap().opt(keep_dims=frozenset({0}))
            )
            for n in range(NT):
                wT = wT_pool.tile([128, KT, 512], mybir.dt.float16, name=f"wT{it}_{n}", tag="wT")
                nc.sync.dma_start_transpose(
                    wT[:], w_re[bass.ds(n * 512, 512), :].opt(keep_dims=frozenset({0}))
                )
                pss = [
                    psum_pool.tile(
                        [128, 512], mybir.dt.float32, name=f"ps{it}_{n}_{m}", tag=f"ps{m}"
                    )
                    for m in range(MT)
                ]
                # k-contiguous per PSUM bank: 32 back-to-back MMs per bank.
                for m in range(MT):
                    for k in range(KT):
                        nc.tensor.matmul(
                            pss[m][:],
                            xT[:, k, bass.ds(m * 128, 128)],
                            wT[:, k, :],
                            start=(k == 0),
                            stop=(k == KT - 1),
                        )
                for m in range(MT):
                    ot = out_pool.tile([128, 512], mybir.dt.float16, name=f"ot{it}_{n}_{m}", tag="ot")
                    nc.vector.tensor_tensor(
                        ot[:],
                        pss[m][:],
                        bias_rep[:, bass.ds(n * 512, 512)],
                        mybir.AluOpType.add,
                    )
                    nc.sync.dma_start(y_re[:, m, bass.ds(n * 512, 512)], ot[:])

    nc.compile()
    return nc


def _build(chain=1):
    nc = bacc.Bacc(
        "TRN2",
        target_bir_lowering=False,
        debug=False,
        num_devices=N_CORES,
    )
    x = nc.dram_tensor("input", [S_SHARD, D_IN], mybir.dt.float16, kind="ExternalInput")
    w = nc.dram_tensor("weight", [D_OUT, D_IN], mybir.dt.float16, kind="ExternalInput")
    b = nc.dram_tensor("bias", [1, D_OUT], mybir.dt.float16, kind="ExternalInput")
    y = nc.dram_tensor("out", [S_SHARD, D_OUT], mybir.dt.float16, kind="ExternalOutput")

    with tile.TileContext(nc) as tc:
        with tc.tile_pool(name="bias_pool", bufs=1) as bias_pool:
            bias_rep = bias_pool.tile([128, D_OUT], mybir.dt.float16)
            # Replicate the bias row into all 128 partitions once.
            nc.sync.dma_start(bias_rep[:], b.ap().to_broadcast((128, D_OUT)))

            def add_bias(nc_, sbuf, md, _data):
                start = md.n_tile_idx * md.n_tile
                nsz = sbuf.shape[-1]
                nc_.vector.tensor_tensor(
                    sbuf,
                    sbuf,
                    bias_rep[:, None, start : start + nsz].to_broadcast(sbuf.shape),
                    mybir.AluOpType.add,
                )

            for it in range(chain):
                if it > 0:
                    # Serialize benchmark iterations so T(chain)-T(1) is an
                    # honest per-execution time.
                    tc.strict_bb_all_engine_barrier()
                matmul_tile_kernel(
                    tc,
                    x.ap(),  # kxm: X.T via DMA transpose -> psum partitions = tokens
                    w.ap(),  # kxn: W.T via DMA transpose -> free dim = out_features
                    y.ap(),
                    transpose_kxm=True,
                    transpose_kxn=True,
                    post_mxn_tile_fn=add_bias,
                    MAX_K_TILE_SIZE=4096,
                )

    nc.compile()
    return nc


def _make_sharded(nc):
    """Build a jitted shard_map fn executing nc's NEFF once on 8 cores.
    Returns (fn, in_names, out_names, out_avals, n_params)."""
    import jax
    from jax.sharding import Mesh, PartitionSpec
    from jax.experimental.shard_map import shard_map

    from concourse import bass2jax, mybir as _mybir

    bass2jax.install_neuronx_cc_hook()

    partition_name = nc.partition_id_tensor.name if nc.partition_id_tensor else None
    in_names, out_names, out_avals = [], [], []
    for alloc in nc.m.functions[0].allocations:
        if not isinstance(alloc, _mybir.MemoryLocationSet):
            continue
        name = alloc.memorylocations[0].name
        if alloc.kind == "ExternalInput":
            if name != partition_name:
                in_names.append(name)
        elif alloc.kind == "ExternalOutput":
            out_names.append(name)
            shape = tuple(alloc.tensor_shape)
            dtype = _mybir.dt.np(alloc.dtype)
            out_avals.append(jax.core.ShapedArray(shape, dtype))
    n_params = len(in_names)
    all_in_names = in_names + out_names
    if partition_name is not None:
        all_in_names.append(partition_name)

    def _body(*args):
        operands = list(args)
        if partition_name is not None:
            operands.append(bass2jax.partition_id_tensor())
        return tuple(
            bass2jax._bass_exec_p.bind(
                *operands,
                out_avals=tuple(out_avals),
                in_names=tuple(all_in_names),
                out_names=tuple(out_names),
                lowering_input_output_aliases=(),
                sim_require_finite=True,
                sim_require_nnan=True,
                nc=nc,
            )
        )

    devices = jax.devices()[:N_CORES]
    mesh = Mesh(np.asarray(devices), ("core",))
    n_outs = len(out_names)
    in_specs = (PartitionSpec("core"),) * (n_params + n_outs)
    out_specs = (PartitionSpec("core"),) * n_outs
    fn = jax.jit(
        shard_map(
            _body, mesh=mesh, in_specs=in_specs, out_specs=out_specs, check_rep=False
        ),
        keep_unused=True,
    )
    return fn, in_names, out_names, out_avals, n_params


BUILDER = None  # set after definitions; defaults to _build_v2


def benchmark(input, weight, bias, iters=12, reps=6):
    """Marginal per-GEMM time: compares a NEFF containing `iters` chained
    (barrier-separated) copies of the kernel against the 1-copy NEFF.
    per_exec = (T(iters) - T(1)) / (iters - 1), min over reps.
    Returns (per_exec_seconds, outputs_list from the 1-copy NEFF).
    """
    import time

    import jax
    from jax.sharding import Mesh, NamedSharding, PartitionSpec

    bld = BUILDER or _build_v2
    if "nc" not in _CACHE:
        _CACHE["nc"] = bld()
    nc1 = _CACHE["nc"]
    key = f"nc_chain{iters}"
    if key not in _CACHE:
        _CACHE[key] = bld(chain=iters)
    ncK = _CACHE[key]

    X = np.ascontiguousarray(np.asarray(input, dtype=np.float16).reshape(S_TOTAL, D_IN))
    Wm = np.ascontiguousarray(np.asarray(weight, dtype=np.float16))
    bm = np.ascontiguousarray(np.asarray(bias, dtype=np.float16).reshape(1, D_OUT))
    in_maps = [
        {
            "input": np.ascontiguousarray(X[i * S_SHARD : (i + 1) * S_SHARD]),
            "weight": Wm,
            "bias": bm,
        }
        for i in range(N_CORES)
    ]

    fn1, in_names, out_names, out_avals, n_params = _make_sharded(nc1)
    fnK = _make_sharded(ncK)[0]

    concat_in = [
        np.concatenate([np.asarray(in_maps[c][nm]) for c in range(N_CORES)], axis=0)
        for nm in in_names
    ]
    concat_zeros = [
        np.zeros((N_CORES * a.shape[0], *a.shape[1:]), a.dtype) for a in out_avals
    ]
    mesh = Mesh(np.asarray(jax.devices()[:N_CORES]), ("core",))
    sh = NamedSharding(mesh, PartitionSpec("core"))
    dev_in = [jax.device_put(a, sh) for a in concat_in]
    dev_zero = [jax.device_put(a, sh) for a in concat_zeros]

    # Warmup both (compiles wrapper + NEFF)
    outs = fn1(*dev_in, *dev_zero)
    jax.block_until_ready(outs)
    outsK = fnK(*dev_in, *dev_zero)
    jax.block_until_ready(outsK)

    best = float("inf")
    t1s, tKs = [], []
    for _ in range(reps):
        t0 = time.perf_counter()
        o1 = fn1(*dev_in, *dev_zero)
        jax.block_until_ready(o1)
        t1 = time.perf_counter()
        oK = fnK(*dev_in, *dev_zero)
        jax.block_until_ready(oK)
        t2 = time.perf_counter()
        t1s.append(t1 - t0)
        tKs.append(t2 - t1)
    per_exec = (min(tKs) - min(t1s)) / (iters - 1)
    print(f"[bench] T1 min={min(t1s)*1e3:.3f} ms  TK min={min(tKs)*1e3:.3f} ms  "
          f"(iters={iters})")

    out_np = [np.asarray(o) for o in outs]
    results = [
        {
            nm: out_np[i].reshape(N_CORES, *out_avals[i].shape)[c]
            for i, nm in enumerate(out_names)
        }
        for c in range(N_CORES)
    ]
    return per_exec, results


def kernel(input, weight, bias):
    global LAST_RESULTS
    bld = BUILDER or _build_v2
    if "nc" not in _CACHE:
        _CACHE["nc"] = bld()
    nc = _CACHE["nc"]

    X = np.ascontiguousarray(np.asarray(input, dtype=np.float16).reshape(S_TOTAL, D_IN))
    Wm = np.ascontiguousarray(np.asarray(weight, dtype=np.float16))
    bm = np.ascontiguousarray(np.asarray(bias, dtype=np.float16).reshape(1, D_OUT))

    in_maps = [
        {
            "input": np.ascontiguousarray(X[i * S_SHARD : (i + 1) * S_SHARD]),
            "weight": Wm,
            "bias": bm,
        }
        for i in range(N_CORES)
    ]

    kwargs = {}
    if TRACE:
        kwargs = dict(trace=True, tmpdir=TRACE_DIR)
    res = run_bass_kernel_spmd(nc, in_maps, list(range(N_CORES)), **kwargs)
    LAST_RESULTS = res

    Y = np.concatenate([res.results[i]["out"] for i in range(N_CORES)], axis=0)
    return Y.reshape(B, S, D_OUT)
